# revision 1
# baseline (speedup 1.0000x reference)
"""Trainium2 Bass kernel for the hypernet-Conv3D module.

Strategy (data-parallel over batch, one sample per NeuronCore):
  - The tiny hypernet MLP (~2 MFLOP vs 58 GFLOP for the conv) runs on the
    host in fp32 numpy; it produces per-sample conv weights [32,16,3,3,3] and
    biases [32], repacked into matmul-ready block-Toeplitz layouts (bf16).
  - The 3D conv runs on device as an implicit GEMM ("Toeplitz-D"):
      * x (host-transposed to [d, cin, h, w] bf16) is processed in 16 windows
        of 4 output d-planes; each window holds 6 input planes (halo 1) in
        SBUF as [96 = 6 planes x 16 cin, 66x66 zero-padded hw] - one
        contiguous-source DMA per window.
      * PSUM tile [128 = 4 outplanes x 32 cout, N cols]; 9 accumulating bf16
        matmuls (one per (kh,kw) offset, applied as a free-dim shift of the
        rhs AP). kd offsets ride inside the block-Toeplitz lhsT [96, 128].
      * Chunks are row-aligned (7 padded rows = 462 cols, final 1-row chunk)
        so the ScalarE PSUM->SBUF evacuation (bias fused) also compacts the
        66-wide padded rows to 64-wide valid rows.
      * One contiguous 2 MB DMA per window writes [4 planes x 32 cout] back.
"""

import os as _os

import numpy as np
import ml_dtypes

import concourse.bacc as bacc
import concourse.mybir as mybir
from concourse.tile import TileContext
from concourse.bass_utils import run_bass_kernel_spmd

B, CIN, COUT, K = 8, 16, 32, 3
D = H = W = 64
NUM_W = CIN * COUT * K**3  # 13824

PW = W + 2          # 66
P2 = PW * PW        # 4356 padded plane
MARGIN = PW + 1     # 67 >= max |(kh-1)*66 + (kw-1)|
XFREE = P2 + 2 * MARGIN  # 4490
GD = 4              # output d-planes per window
NWIN = D // GD      # 16
NPL = GD + 2        # input planes per window
N_CORES = 8
NXBUF = 5           # rotating input-window buffers (deep prefetch)
WARM_MM = int(_os.environ.get("CONV_WARM_MM", "6"))  # warm-up matmuls
# (start_padded_row, n_rows) chunks covering padded rows 1..64
ROW_CHUNKS = [(1 + 7 * i, 7) for i in range(9)] + [(64, 1)]

f32 = mybir.dt.float32
bf16 = mybir.dt.bfloat16

# matmul datapath: "bf16" (half bytes, ~2e-3 rel err) or "f32r" (fp32
# storage, PE fast-fp32 mode, near-fp32 accuracy)
MM_MODE = _os.environ.get("CONV_MM_MODE", "bf16")
if MM_MODE == "bf16":
    DT = bf16
    NPDT = ml_dtypes.bfloat16
    MM_CAST = None
    ODT = bf16
    NPODT = ml_dtypes.bfloat16
else:
    # float32r end-to-end: walrus requires matmul operands to be f32r-typed
    # at the producer, so the DRAM tensors and SBUF tiles carry the dtype.
    DT = mybir.dt.float32r
    NPDT = np.float32
    MM_CAST = None
    ODT = f32
    NPODT = np.float32


# ---------------------------------------------------------------- host side

def _host_hypernet(inputs):
    f = np.asarray(inputs["features"], np.float32)
    fc0_w = np.asarray(inputs["fc0_w"], np.float32)
    fc0_b = np.asarray(inputs["fc0_b"], np.float32)
    fc1_w = np.asarray(inputs["fc1_w"], np.float32)
    fc1_b = np.asarray(inputs["fc1_b"], np.float32)
    a0 = np.float32(np.asarray(inputs["a0"]).reshape(-1)[0])
    a1 = np.float32(np.asarray(inputs["a1"]).reshape(-1)[0])
    wg_w = np.asarray(inputs["wg_w"], np.float32)
    wg_b = np.asarray(inputs["wg_b"], np.float32)
    h = f @ fc0_w.T + fc0_b
    h = np.where(h >= 0, h, a0 * h)
    h = h @ fc1_w.T + fc1_b
    h = np.where(h >= 0, h, a1 * h)
    params = h @ wg_w.T + wg_b
    w = params[:, :NUM_W].reshape(B, COUT, CIN, K, K, K).astype(np.float32)
    bias = params[:, NUM_W:].astype(np.float32)
    return w, bias


def _build_wmat(w):
    """w: [32,16,3,3,3] -> [96, 9*128] block-Toeplitz lhsT bank (bf16).

    Column block i = 3*kh + kw holds lhsT_i with
      lhsT_i[16*pl + cin, 32*j + c] = w[c, cin, pl - j, kh, kw]  (0 <= pl-j <= 2)
    """
    wmat = np.zeros((9, 96, 128), np.float32)
    wt = np.transpose(w, (3, 4, 1, 0, 2))  # [kh, kw, cin, cout, kd]
    for kh in range(3):
        for kw in range(3):
            i = 3 * kh + kw
            for j in range(GD):
                for kd in range(3):
                    pl = j + kd
                    wmat[i, 16 * pl:16 * pl + 16, 32 * j:32 * j + 32] = \
                        wt[kh, kw, :, :, kd]
    return np.ascontiguousarray(
        wmat.transpose(1, 0, 2).reshape(96, 9 * 128).astype(NPDT))


# -------------------------------------------------------------- device side

def _conv_body(tc, xt_d, wm_d, bias_d, y_d):
    nc = tc.nc
    with (
        tc.tile_pool(name="const", bufs=1) as cpool,
        tc.tile_pool(name="xw", bufs=1) as xpool,
        tc.tile_pool(name="osb", bufs=4) as opool,
        tc.tile_pool(name="ps", bufs=8, space="PSUM") as pspool,
    ):
        # PE warm-up: dependency-free matmuls on a scratch tile ramp the PE
        # p-state to max and keep the engine busy while the first input DMA
        # lands, so real matmuls start at full clock with no idle gap.
        warm = cpool.tile([96, 512], DT, name="warm")
        _wm = warm[:, :]
        if DT == mybir.dt.float32r:
            _wm = _wm.bitcast(f32)
        nc.gpsimd.memset(_wm, 0.0)
        for i in range(WARM_MM):
            wps = pspool.tile([128, 512], f32, name="wps", tag="ps")
            nc.tensor.matmul(
                wps[:, :512],
                lhsT=warm[:, 0:128],
                rhs=warm[:, 0:512],
                start=True,
                stop=True,
            )

        wsb = cpool.tile([96, 9 * 128], DT, name="wsb")
        nc.sync.dma_start(out=wsb, in_=wm_d[:, :])
        bsb = cpool.tile([128, 1], f32, name="bsb")
        # issued from Act (idle at startup) to keep SP's chain short
        nc.scalar.dma_start(out=bsb, in_=bias_d[:, :])

        # persistent rotating window tiles; the DMA rewrites the whole
        # [MARGIN, MARGIN+P2) span every window (host planes are pre-padded,
        # with zero planes at d=-1 and d=64), so only the margins need
        # zeroing once at startup.
        def _memset0(ap):
            # Memset of an f32r-typed AP is invalid ISA; zero via an f32 view.
            if DT == mybir.dt.float32r:
                ap = ap.bitcast(f32)
            nc.gpsimd.memset(ap, 0.0)

        xwins = []
        for i in range(NXBUF):
            t = xpool.tile([96, XFREE], DT, name=f"xwin{i}", tag=f"xwin{i}")
            _memset0(t[:, 0:MARGIN])
            _memset0(t[:, MARGIN + P2:XFREE])
            xwins.append(t)

        for win in range(NWIN):
            d0 = GD * win
            xw = xwins[win % NXBUF]
            # input planes d0-1 .. d0+4 (zero-plane padded: src idx d0..d0+6)
            src = xt_d[d0:d0 + NPL].rearrange("d c n -> (d c) n")
            if win == 0:
                # split: first 16 padded rows land early so chunk 0/1
                # matmuls can start while the rest streams in.
                cut = 16 * PW
                nc.sync.dma_start(out=xw[:, MARGIN:MARGIN + cut],
                                  in_=src[:, 0:cut])
                nc.sync.dma_start(out=xw[:, MARGIN + cut:MARGIN + P2],
                                  in_=src[:, cut:])
            else:
                nc.sync.dma_start(out=xw[:, MARGIN:MARGIN + P2], in_=src)

            osb = opool.tile([128, D * W], ODT, name="osb", tag="osb")
            # last window: small final chunks + piecewise flush right after
            # the chunks producing them, so the critical-tail DMA is tiny.
            dst = y_d[:, d0:d0 + GD].rearrange("c j h w -> j c (h w)")
            if win == NWIN - 1:
                chunks = [(1 + 7 * i, 7) for i in range(8)] + \
                    [(57, 3), (60, 3), (63, 2)]
                pieces = {4: (0, 2240), 6: (2240, 3136), 8: (3136, 3776),
                          9: (3776, 3968), 10: (3968, 4096)}
            else:
                chunks = ROW_CHUNKS
                pieces = {}
            for ci, (r0, nr) in enumerate(chunks):
                n = nr * W
                ps = pspool.tile([128, 512], f32, name="ps", tag="ps")
                # per-row 64-col matmuls: no pad columns charged
                for r in range(nr):
                    for i in range(9):
                        kh, kw = divmod(i, 3)
                        delta = (kh - 1) * PW + (kw - 1)
                        rs = MARGIN + (r0 + r) * PW + 1 + delta
                        nc.tensor.matmul(
                            ps[:, W * r:W * (r + 1)],
                            lhsT=wsb[:, 128 * i:128 * (i + 1)],
                            rhs=xw[:, rs:rs + W],
                            start=(i == 0),
                            stop=(i == 8),
                        )
                # PSUM -> SBUF: fused bias add (rows already compact)
                nc.scalar.activation(
                    out=osb[:, (r0 - 1) * W:(r0 - 1 + nr) * W],
                    in_=ps[:, :n],
                    func=mybir.ActivationFunctionType.Identity,
                    bias=bsb[:, 0:1],
                    scale=1.0,
                )
                if win == NWIN - 1 and ci in pieces:
                    a, b = pieces[ci]
                    nc.scalar.dma_start(out=dst[:, :, a:b], in_=osb[:, a:b])

            # one contiguous DMA: partition 32j+c -> y[c, d0+j, :, :]
            # issued from the Act engine so the SP sequencer (input
            # prefetches) never blocks on this window's compute.
            if win != NWIN - 1:
                nc.scalar.dma_start(out=dst, in_=osb[:, :])


_NC_CACHE = {}


def _get_nc():
    if "nc" not in _NC_CACHE:
        nc = bacc.Bacc("TRN2", target_bir_lowering=False, debug=False)
        xt_d = nc.dram_tensor("xt", [D + 2, CIN, P2], DT, kind="ExternalInput")
        wm_d = nc.dram_tensor("wmat", [96, 9 * 128], DT, kind="ExternalInput")
        bias_d = nc.dram_tensor("bias", [128, 1], f32, kind="ExternalInput")
        y_d = nc.dram_tensor("y", [COUT, D, H, W], ODT, kind="ExternalOutput")
        with TileContext(nc) as tc:
            _conv_body(tc, xt_d, wm_d, bias_d, y_d)
        nc.finalize()  # runs Bacc regalloc/DCE passes, then freezes
        _NC_CACHE["nc"] = nc
    return _NC_CACHE["nc"]


def _run(inputs, trace=False):
    w, bias = _host_hypernet(inputs)
    x = np.asarray(inputs["x"], np.float32)
    in_maps = []
    for b in range(B):
        # [d+2, cin, 66, 66] zero-padded planes (incl. zero d=-1/d=64
        # boundary planes), flattened to [d+2, cin, 4356]
        xt = np.zeros((D + 2, CIN, PW, PW), NPDT)
        xt[1:1 + D, :, 1:1 + H, 1:1 + W] = np.transpose(x[b], (1, 0, 2, 3))
        xt = np.ascontiguousarray(xt.reshape(D + 2, CIN, P2))
        in_maps.append({
            "xt": xt,
            "wmat": _build_wmat(w[b]),
            "bias": np.ascontiguousarray(np.tile(bias[b], GD).reshape(128, 1)),
        })
    nc = _get_nc()
    res = run_bass_kernel_spmd(
        nc, in_maps, core_ids=list(range(N_CORES)), trace=trace,
    )
    y = np.stack([np.asarray(res.results[b]["y"], np.float32) for b in range(B)])
    return y, res


def kernel(**inputs) -> np.ndarray:
    y, _ = _run(inputs, trace=False)
    return y



# revision 2
# speedup vs baseline: 1.4839x; 1.4839x over previous
"""Trainium2 Bass kernel for the hypernet-Conv3D module.

Strategy (data-parallel over batch, one sample per NeuronCore):
  - The tiny hypernet MLP (~2 MFLOP vs 58 GFLOP for the conv) runs on the
    host in fp32 numpy; it produces per-sample conv weights [32,16,3,3,3] and
    biases [32], repacked into matmul-ready block-Toeplitz layouts (bf16).
  - The 3D conv runs on device as an implicit GEMM with a 2x2 (d,h) output
    stack ("Toeplitz-DH"):
      * Output partition dim = 128 = 2 d-planes x 2 h-rows x 32 cout.
      * Contraction = 4 input planes x 4 input h-rows x 16 cin = 256 w-lines
        = exactly two 128-row tiles; each tile is read with 3 w-shifts
        (kw taps as free-dim AP offsets). 6 accumulating matmuls replace the
        previous scheme's 9 -> 1.5x fewer PE columns (56.25% vs 37.5% MAC
        packing; PE cost is output-cols only, independent of row count).
      * Host pre-packs x into the exact SBUF tile layout
        xt[dp, (hr2, pl4, cin16), (t33, q66)] (d/h/w all zero-padded, w-lines
        66 wide) so each d-pair window is ONE fully contiguous 557 KB DMA.
      * Per d-pair: 4 PSUM banks of [128, 512] (8 h-pair blocks each); each
        gets 6 matmuls of 512 cols (rhs free AP = (8 blocks, 64 w)); ScalarE
        evacuates with fused bias; one contiguous 512 KB output DMA.
      * Output leaves the device in (dp, (jd,jh,c), (t,w)) order; the host
        un-permutes to [cout, d, h, w] for free.
"""

import os as _os

import numpy as np
import ml_dtypes

import concourse.bacc as bacc
import concourse.mybir as mybir
from concourse.tile import TileContext
from concourse.bass_utils import run_bass_kernel_spmd

B, CIN, COUT, K = 8, 16, 32, 3
D = H = W = 64
NUM_W = CIN * COUT * K**3  # 13824

QW = W + 2          # 66: padded w-line length
NDP = D // 2        # 32 d-pair windows
NT = H // 2 + 1     # 33 h-pair tiles per window (halo 1 row each side)
XFREE = NT * QW     # 2178
NBLK = H // 2       # 32 h-pair output blocks per window
GRP = 8             # h-pair blocks per PSUM bank (8 * 64 = 512 cols)
NGRP = NBLK // GRP  # 4 PSUM groups per window
OFREE = NBLK * W    # 2048 output cols per window
N_CORES = 8
NXBUF = 4           # rotating input-window buffers (deep prefetch)
WARM_MM = int(_os.environ.get("CONV_WARM_MM", "5"))  # warm-up matmuls

f32 = mybir.dt.float32
bf16 = mybir.dt.bfloat16

DT = bf16
NPDT = ml_dtypes.bfloat16
ODT = bf16


# ---------------------------------------------------------------- host side

def _host_hypernet(inputs):
    f = np.asarray(inputs["features"], np.float32)
    fc0_w = np.asarray(inputs["fc0_w"], np.float32)
    fc0_b = np.asarray(inputs["fc0_b"], np.float32)
    fc1_w = np.asarray(inputs["fc1_w"], np.float32)
    fc1_b = np.asarray(inputs["fc1_b"], np.float32)
    a0 = np.float32(np.asarray(inputs["a0"]).reshape(-1)[0])
    a1 = np.float32(np.asarray(inputs["a1"]).reshape(-1)[0])
    wg_w = np.asarray(inputs["wg_w"], np.float32)
    wg_b = np.asarray(inputs["wg_b"], np.float32)
    h = f @ fc0_w.T + fc0_b
    h = np.where(h >= 0, h, a0 * h)
    h = h @ fc1_w.T + fc1_b
    h = np.where(h >= 0, h, a1 * h)
    params = h @ wg_w.T + wg_b
    w = params[:, :NUM_W].reshape(B, COUT, CIN, K, K, K).astype(np.float32)
    bias = params[:, NUM_W:].astype(np.float32)
    return w, bias


def _build_wmat(w):
    """w: [32,16,3,3,3] -> [128, 6*128] block-Toeplitz lhsT bank (bf16).

    Column block m = 3*role + kw holds lhsT_m with
      lhsT_m[64*hr + 16*pl + cin, 64*jd + 32*jh + c] = w[c, cin, kd, kh, kw]
    where kd = pl - jd (0..2 valid) and kh = 2*role + hr - jh (0..2 valid);
    invalid slots are zero.
    """
    wmat = np.zeros((128, 6 * 128), np.float32)
    for role in range(2):
        for kw in range(3):
            m = 3 * role + kw
            for hr in range(2):
                for pl in range(4):
                    row = 64 * hr + 16 * pl
                    for jd in range(2):
                        kd = pl - jd
                        if not 0 <= kd <= 2:
                            continue
                        for jh in range(2):
                            kh = 2 * role + hr - jh
                            if not 0 <= kh <= 2:
                                continue
                            col = m * 128 + 64 * jd + 32 * jh
                            wmat[row:row + 16, col:col + 32] = \
                                w[:, :, kd, kh, kw].T
    return np.ascontiguousarray(wmat.astype(NPDT))


def _build_xt(xb):
    """xb: [16,64,64,64] f32 -> [32*128, 2178] bf16 tile-layout input.

    xt[128*dp + 64*hr + 16*pl + cin, 66*t + q] =
        xpad[2*dp + pl, 2*t + hr, cin, q]
    with xpad = x zero-padded by 1 in d, h, w (layout [d,h,cin,w]).
    """
    xp = np.zeros((66, 66, CIN, QW), np.float32)
    xp[1:65, 1:65, :, 1:65] = np.transpose(xb, (1, 2, 0, 3))
    d_idx = 2 * np.arange(NDP)[:, None] + np.arange(4)[None, :]   # [32,4]
    h_idx = 2 * np.arange(NT)[:, None] + np.arange(2)[None, :]    # [33,2]
    # fancy-index -> [dp, pl, t, hr, cin, q]
    arr = xp[d_idx[:, :, None, None], h_idx[None, None, :, :]]
    # -> [dp, hr, pl, cin, t, q]
    arr = np.ascontiguousarray(arr.transpose(0, 3, 1, 4, 2, 5))
    return arr.reshape(NDP * 128, XFREE).astype(NPDT)


def _unpack_y(yd):
    """yd: [32*128, 2048] -> [32, 64, 64, 64] f32.

    yd[128*dp + 64*jd + 32*jh + c, 64*t + w] = y[c, 2*dp+jd, 2*t+jh, w]
    """
    a = np.asarray(yd, np.float32).reshape(NDP, 2, 2, COUT, NBLK, W)
    return a.transpose(3, 0, 1, 4, 2, 5).reshape(COUT, D, H, W)


# -------------------------------------------------------------- device side

def _conv_body(tc, xt_d, wm_d, bias_d, y_d):
    nc = tc.nc
    with (
        tc.tile_pool(name="const", bufs=1) as cpool,
        tc.tile_pool(name="xw", bufs=1) as xpool,
        tc.tile_pool(name="osb", bufs=3) as opool,
        tc.tile_pool(name="ps", bufs=8, space="PSUM") as pspool,
    ):
        # PE warm-up: dependency-free matmuls on a scratch tile ramp the PE
        # p-state and keep the engine busy while the first input DMA lands.
        warm = cpool.tile([128, 512], DT, name="warm")
        nc.gpsimd.memset(warm[:, :], 0.0)
        for i in range(WARM_MM):
            wps = pspool.tile([128, 512], f32, name="wps", tag="ps")
            nc.tensor.matmul(
                wps[:, :512],
                lhsT=warm[:, 0:128],
                rhs=warm[:, 0:512],
                start=True,
                stop=True,
            )

        wsb = cpool.tile([128, 6 * 128], DT, name="wsb")
        nc.sync.dma_start(out=wsb, in_=wm_d[:, :])
        bsb = cpool.tile([128, 1], f32, name="bsb")
        # issued from Act (idle at startup) to keep SP's chain short
        nc.scalar.dma_start(out=bsb, in_=bias_d[:, :])

        xwins = [
            xpool.tile([128, XFREE], DT, name=f"xwin{i}", tag=f"xwin{i}")
            for i in range(NXBUF)
        ]

        for dp in range(NDP):
            xw = xwins[dp % NXBUF]
            src = xt_d[128 * dp:128 * (dp + 1)]
            if dp == 0:
                # split: the first PSUM group's tiles (t 0..8) land early so
                # matmuls start while the rest streams in.
                cut = (GRP + 1) * QW
                nc.sync.dma_start(out=xw[:, :cut], in_=src[:, :cut])
                nc.sync.dma_start(out=xw[:, cut:], in_=src[:, cut:])
            else:
                nc.sync.dma_start(out=xw, in_=src)
            xv = xw[:, :].rearrange("p (t q) -> p t q", q=QW)

            osb = opool.tile([128, OFREE], ODT, name="osb", tag="osb")
            for g in range(NGRP):
                ps = pspool.tile([128, 512], f32, name="ps", tag="ps")
                i = 0
                for role in range(2):
                    for kw in range(3):
                        rhs = xv[:, GRP * g + role:GRP * g + role + GRP,
                                 kw:kw + W]
                        m = 3 * role + kw
                        nc.tensor.matmul(
                            ps[:, :512],
                            lhsT=wsb[:, 128 * m:128 * (m + 1)],
                            rhs=rhs,
                            start=(i == 0),
                            stop=(i == 5),
                        )
                        i += 1
                # PSUM -> SBUF: fused bias add
                nc.scalar.activation(
                    out=osb[:, 512 * g:512 * (g + 1)],
                    in_=ps[:, :512],
                    func=mybir.ActivationFunctionType.Identity,
                    bias=bsb[:, 0:1],
                    scale=1.0,
                )
            # one contiguous DMA per window, issued from Act so the SP
            # sequencer (input prefetches) never blocks on compute.
            nc.scalar.dma_start(out=y_d[128 * dp:128 * (dp + 1)], in_=osb[:, :])


_NC_CACHE = {}


def _get_nc():
    if "nc" not in _NC_CACHE:
        nc = bacc.Bacc("TRN2", target_bir_lowering=False, debug=False)
        xt_d = nc.dram_tensor("xt", [NDP * 128, XFREE], DT, kind="ExternalInput")
        wm_d = nc.dram_tensor("wmat", [128, 6 * 128], DT, kind="ExternalInput")
        bias_d = nc.dram_tensor("bias", [128, 1], f32, kind="ExternalInput")
        y_d = nc.dram_tensor("y", [NDP * 128, OFREE], ODT, kind="ExternalOutput")
        with TileContext(nc) as tc:
            _conv_body(tc, xt_d, wm_d, bias_d, y_d)
        nc.finalize()  # runs Bacc regalloc/DCE passes, then freezes
        _NC_CACHE["nc"] = nc
    return _NC_CACHE["nc"]


def _run(inputs, trace=False):
    w, bias = _host_hypernet(inputs)
    x = np.asarray(inputs["x"], np.float32)
    in_maps = []
    for b in range(B):
        in_maps.append({
            "xt": _build_xt(x[b]),
            "wmat": _build_wmat(w[b]),
            "bias": np.ascontiguousarray(np.tile(bias[b], 4).reshape(128, 1)),
        })
    nc = _get_nc()
    res = run_bass_kernel_spmd(
        nc, in_maps, core_ids=list(range(N_CORES)), trace=trace,
    )
    y = np.stack([_unpack_y(res.results[b]["y"]) for b in range(B)])
    return y, res


def kernel(**inputs) -> np.ndarray:
    y, _ = _run(inputs, trace=False)
    return y
